# revision 11
# baseline (speedup 1.0000x reference)
"""Multi-head attention (B=2, S=2048, D=1024, H=16, d=64) on 8 TRN2 NeuronCores.

Sharding: core i handles batch b = i // 4 and query rows [qb*512, (qb+1)*512)
with qb = i % 4. No collectives: each core computes K/V for its whole batch,
attention for its query block, and the full output projection for its rows.

Device-side layout (everything "transposed" so no on-chip transposes needed):
  - host feeds embT = embedding[b].T as bf16  [D, S]
  - Q^T, K^T computed per head-pair via block-diagonal packed weights
    (lhsT = blkdiag(W_h0^T, W_h1^T), rhs = embT rows of the pair)
  - S^T = K^T-chunks (stationary) x Q^T (moving)  ->  [t, s] layout in PSUM
  - exp on ScalarE (no max subtraction; scores are in [-11, 11])
  - V is stored [t, e] with a ones-column appended; A*V matmul then yields
    both the unnormalized output (rows 0:64) and Z = sum_t E (row 64)
  - normalization: reciprocal of Z, broadcast across partitions via a
    stride-0 DMA, multiplied in during PSUM evacuation
  - out-projection: head pairs stacked along K=128, accumulated over all
    8 pairs in PSUM; bv/bo folded into a host-precomputed bias bo2
"""

import math
from contextlib import ExitStack
from functools import lru_cache

import ml_dtypes
import numpy as np

import concourse.bass as bass
import concourse.bacc as bacc
import concourse.mybir as mybir
import concourse.tile as tile

BF16 = mybir.dt.bfloat16
F32 = mybir.dt.float32
NPBF16 = ml_dtypes.bfloat16

B, S, D, H, d = 2, 2048, 1024, 16, 64
NCORES = 8
QBLKS = 4              # query blocks per batch
QB = S // QBLKS        # 512 query rows per core
NP = H // 2            # 8 head pairs
TCH = S // 128         # 16 t-chunks of 128
SCALE = 1.0 / math.sqrt(d)
EXP = mybir.ActivationFunctionType.Exp


def build_nc() -> bass.Bass:
    nc = bacc.Bacc("TRN2", target_bir_lowering=False, debug=False)

    xT_d = nc.dram_tensor("xT", [D, S], BF16, kind="ExternalInput").ap()
    xTqa_d = nc.dram_tensor("xTqa", [H, 65, QB], BF16, kind="ExternalInput").ap()
    wqa_d = nc.dram_tensor("wqa", [H, 65, 64], BF16, kind="ExternalInput").ap()
    wk_d = nc.dram_tensor("wk_blk", [NP, 128, 128], BF16, kind="ExternalInput").ap()
    wv_d = nc.dram_tensor("wv_blk", [NP, 128, 128], BF16, kind="ExternalInput").ap()
    woT_d = nc.dram_tensor("woT", [D, D], BF16, kind="ExternalInput").ap()
    bo2_d = nc.dram_tensor("bo2", [1, D], F32, kind="ExternalInput").ap()
    y_d = nc.dram_tensor("y", [QB, D], F32, kind="ExternalOutput").ap()

    rdr_d = nc.dram_tensor("rscratch", [NP, 2, QB], F32).ap()

    with ExitStack() as ctx:
        tc = ctx.enter_context(tile.TileContext(nc))
        persist = ctx.enter_context(tc.tile_pool(name="persist", bufs=1))

        wqa_sb = [persist.tile([65, 64], BF16, name=f"wqa{h}", tag=f"wqa{h}") for h in range(H)]
        wk_sb = [persist.tile([128, 128], BF16, name=f"wk{p}", tag=f"wk{p}") for p in range(NP)]
        wv_sb = [persist.tile([128, 128], BF16, name=f"wv{p}", tag=f"wv{p}") for p in range(NP)]
        xTqa_sb = [persist.tile([65, QB], BF16, name=f"xTqa{h}", tag=f"xTqa{h}") for h in range(H)]
        boB_sb = persist.tile([128, D], F32, name="boB", tag="boB")
        woT_sb = [persist.tile([128, D], BF16, name=f"woT{p}", tag=f"woT{p}") for p in range(NP)]
        qT_sb = [persist.tile([128, QB], BF16, name=f"qT{p}", tag=f"qT{p}") for p in range(NP)]
        kT_sb = [persist.tile([128, S], BF16, name=f"kT{p}", tag=f"kT{p}") for p in range(NP)]
        vv_sb = [
            [persist.tile([128, 130], BF16, name=f"vv{p}_{t}", tag=f"vv{p}_{t}") for t in range(TCH)]
            for p in range(NP)
        ]
        outT_sb = [persist.tile([128, QB], BF16, name=f"outT{p}", tag=f"outT{p}") for p in range(NP)]

        for p in range(NP):
            nc.sync.dma_start(out=wk_sb[p][:], in_=wk_d[p])
            nc.sync.dma_start(out=wv_sb[p][:], in_=wv_d[p])
            nc.sync.dma_start(out=woT_sb[p][:], in_=woT_d[p * 128 : (p + 1) * 128, :])
        for h in range(H):
            nc.sync.dma_start(out=wqa_sb[h][:], in_=wqa_d[h])
            nc.sync.dma_start(out=xTqa_sb[h][:], in_=xTqa_d[h])
        # broadcast bo2 [1, D] across all 128 partitions
        nc.sync.dma_start(
            out=boB_sb[:],
            in_=bass.AP(tensor=bo2_d.tensor, offset=bo2_d.offset, ap=[[0, 128], [1, D]]),
        )

        # ---------------- Phase 1: Q^T, K^T, V projections ----------------
        with (
            tc.tile_pool(name="xTpool", bufs=1) as xpool,
            tc.tile_pool(name="pp1", bufs=3, space="PSUM") as pp1,
        ):
            xT_sb = [xpool.tile([128, S], BF16, name=f"xT{p}", tag=f"xT{p}") for p in range(NP)]
            for p in range(NP):
                nc.sync.dma_start(out=xT_sb[p][:], in_=xT_d[p * 128 : (p + 1) * 128, :])

            for p in range(NP):
                # Q~^T per head (bias folded in via ones row): [64 e, QB s]
                for dlt in range(2):
                    h = 2 * p + dlt
                    pq = pp1.tile([64, QB], F32, name="pq", tag="qk")
                    nc.tensor.matmul(pq[:], wqa_sb[h][:], xTqa_sb[h][:], start=True, stop=True)
                    nc.vector.tensor_copy(qT_sb[p][dlt * 64 : (dlt + 1) * 64, :], pq[:])
                # K^T (pair-stacked): [128 e, S t]
                for ck in range(S // 512):
                    pk = pp1.tile([128, 512], F32, name="pk", tag="qk")
                    nc.tensor.matmul(
                        pk[:],
                        wk_sb[p][:],
                        xT_sb[p][:, ck * 512 : (ck + 1) * 512],
                        start=True,
                        stop=True,
                    )
                    nc.vector.tensor_copy(kT_sb[p][:, ck * 512 : (ck + 1) * 512], pk[:])
                # V natural layout [t, e-pair], with ones columns at 64 and 129
                for t in range(TCH):
                    pv = pp1.tile([128, 128], F32, name="pv", tag="v")
                    nc.tensor.matmul(
                        pv[:],
                        xT_sb[p][:, t * 128 : (t + 1) * 128],
                        wv_sb[p][:],
                        start=True,
                        stop=True,
                    )
                    vt = vv_sb[p][t]
                    nc.vector.tensor_copy(vt[:, 0:64], pv[:, 0:64])
                    nc.vector.tensor_copy(vt[:, 65:129], pv[:, 64:128])
                    nc.vector.memset(vt[:, 64:65], 1.0)
                    nc.vector.memset(vt[:, 129:130], 1.0)

        # ---------------- Phase 2: attention (software-pipelined) ----------------
        # PE stream: S-matmuls of group g+1 are emitted BEFORE AV-matmuls of
        # group g, so the PE never drains while ScalarE runs exp(g).
        with (
            tc.tile_pool(name="pps", bufs=2, space="PSUM") as pps,
            tc.tile_pool(name="ppav", bufs=3, space="PSUM") as ppav,
            tc.tile_pool(name="eTp", bufs=4) as eTp,
            tc.tile_pool(name="rbp", bufs=2) as rbp,
        ):
            NG = TCH // 2
            heads = [(p, dlt) for p in range(NP) for dlt in range(2)]
            av_of = {}
            pending = None

            def emit_pair_evac(p):
                Zb = rbp.tile([128, QB], F32, name="Zb", tag="Zb")
                Rb = rbp.tile([128, QB], F32, name="Rb", tag="Rb")
                for dlt in range(2):
                    z2 = rbp.tile([1, QB], F32, name=f"z2{dlt}", tag=f"z2{dlt}")
                    nc.vector.tensor_copy(z2[:], av_of[(p, dlt)][64:65, :])
                    nc.sync.dma_start(out=rdr_d[p, dlt], in_=z2[:])
                    src_ap = rdr_d[p, dlt]
                    nc.sync.dma_start(
                        out=Zb[dlt * 64 : (dlt + 1) * 64, :],
                        in_=bass.AP(
                            tensor=src_ap.tensor,
                            offset=src_ap.offset,
                            ap=[[0, 64], [1, QB]],
                        ),
                    )
                nc.vector.reciprocal(Rb[:], Zb[:])
                for dlt in range(2):
                    nc.vector.tensor_mul(
                        outT_sb[p][dlt * 64 : (dlt + 1) * 64, :],
                        av_of[(p, dlt)][0:64, :],
                        Rb[dlt * 64 : (dlt + 1) * 64, :],
                    )

            def emit_av(p, dlt, g, eT):
                av = av_of[(p, dlt)]
                for j in range(2):
                    t = g * 2 + j
                    nc.tensor.matmul(
                        av[0:65, :],
                        vv_sb[p][t][:, dlt * 65 : dlt * 65 + 65],
                        eT[:, j * 512 : (j + 1) * 512],
                        start=(g == 0 and j == 0),
                        stop=(g == NG - 1 and j == 1),
                    )
                if dlt == 1 and g == NG - 1:
                    emit_pair_evac(p)

            for p, dlt in heads:
                av_of[(p, dlt)] = ppav.tile([128, QB], F32, name="av", tag="av")
                klo = dlt * 64
                for g in range(NG):
                    ps = pps.tile([128, 1024], F32, name="ps", tag="ps")
                    for j in range(2):
                        t = g * 2 + j
                        nc.tensor.matmul(
                            ps[:, j * 512 : (j + 1) * 512],
                            kT_sb[p][klo : klo + 64, t * 128 : (t + 1) * 128],
                            qT_sb[p][klo : klo + 64, :],
                            start=True,
                            stop=True,
                        )
                    eT = eTp.tile([128, 1024], BF16, name="eT", tag="eT")
                    nc.scalar.activation(eT[:], ps[:], EXP, scale=SCALE)
                    if pending is not None:
                        emit_av(*pending)
                    pending = (p, dlt, g, eT)
            emit_av(*pending)

        # ---------------- Phase 3: output projection ----------------
        with (
            tc.tile_pool(name="ppy", bufs=2, space="PSUM") as ppy,
            tc.tile_pool(name="yop", bufs=3) as yop,
        ):
            for sc in range(QB // 128):
                for nk in range(D // 512):
                    py = ppy.tile([128, 512], F32, name="py", tag="py")
                    for p in range(NP):
                        nc.tensor.matmul(
                            py[:],
                            outT_sb[p][:, sc * 128 : (sc + 1) * 128],
                            woT_sb[p][:, nk * 512 : (nk + 1) * 512],
                            start=(p == 0),
                            stop=(p == NP - 1),
                        )
                    yt = yop.tile([128, 512], F32, name="yt", tag="yt")
                    nc.vector.tensor_add(yt[:], py[:], boB_sb[:, nk * 512 : (nk + 1) * 512])
                    nc.sync.dma_start(
                        out=y_d[sc * 128 : (sc + 1) * 128, nk * 512 : (nk + 1) * 512],
                        in_=yt[:],
                    )

    nc.finalize()
    return nc


@lru_cache(maxsize=1)
def _cached_nc() -> bass.Bass:
    return build_nc()


def prepare_in_maps(embedding, Wq, Wk, Wv, bq, bk, bv, Wo, bo):
    """Host-side sharding/packing. Returns per-core input maps."""
    emb = np.asarray(embedding, dtype=np.float32)
    Wq = np.asarray(Wq, dtype=np.float32)
    Wk = np.asarray(Wk, dtype=np.float32)
    Wv = np.asarray(Wv, dtype=np.float32)
    bq = np.asarray(bq, dtype=np.float32)
    bk = np.asarray(bk, dtype=np.float32)
    bv = np.asarray(bv, dtype=np.float32)
    Wo = np.asarray(Wo, dtype=np.float32)
    bo = np.asarray(bo, dtype=np.float32)

    wk_blk = np.zeros([NP, 128, 128], np.float32)
    wv_blk = np.zeros([NP, 128, 128], np.float32)
    for p in range(NP):
        h0, h1 = 2 * p, 2 * p + 1
        wk_blk[p, 0:64, 0:64] = Wk[h0].T
        wk_blk[p, 64:128, 64:128] = Wk[h1].T
        wv_blk[p, 0:64, 0:64] = Wv[h0].T
        wv_blk[p, 64:128, 64:128] = Wv[h1].T
    # per-head augmented Q weights: rows 0:64 = Wq_h^T, row 64 = bq_h
    wqa = np.zeros([H, 65, 64], np.float32)
    for h in range(H):
        wqa[h, 0:64, :] = Wq[h].T
        wqa[h, 64, :] = bq[h]

    wqa16 = wqa.astype(NPBF16)
    wk16 = wk_blk.astype(NPBF16)
    wv16 = wv_blk.astype(NPBF16)
    woT16 = np.ascontiguousarray(Wo.T).astype(NPBF16)
    bo2 = (bo + Wo @ bv.reshape(-1)).reshape(1, D).astype(np.float32)

    xT_by_b = [np.ascontiguousarray(emb[b].T).astype(NPBF16) for b in range(B)]

    in_maps = []
    for core in range(NCORES):
        b, qb = core // QBLKS, core % QBLKS
        xT = xT_by_b[b]
        # per-head augmented xTq: rows 0:64 = embT rows of head h, row 64 = ones
        xTqa = np.ones([H, 65, QB], np.float32)
        for h in range(H):
            xTqa[h, 0:64, :] = xT[h * 64 : (h + 1) * 64, qb * QB : (qb + 1) * QB]
        in_maps.append(
            dict(
                xT=xT,
                xTqa=xTqa.astype(NPBF16),
                wqa=wqa16,
                wk_blk=wk16,
                wv_blk=wv16,
                woT=woT16,
                bo2=bo2,
            )
        )
    return in_maps


def assemble(results) -> np.ndarray:
    out = np.empty([B, S, D], np.float32)
    for core in range(NCORES):
        b, qb = core // QBLKS, core % QBLKS
        out[b, qb * QB : (qb + 1) * QB, :] = results[core]["y"]
    return out


def kernel(**inputs) -> np.ndarray:
    from concourse.bass_utils import run_bass_kernel_spmd

    in_maps = prepare_in_maps(**inputs)
    nc = _cached_nc()
    res = run_bass_kernel_spmd(nc, in_maps, list(range(NCORES)))
    return assemble(res.results)


# revision 13
# speedup vs baseline: 1.2414x; 1.2414x over previous
"""Multi-head attention (B=2, S=2048, D=1024, H=16, d=64) on 8 TRN2 NeuronCores.

Sharding: core i handles batch b = i // 4 and query rows [qb*512, (qb+1)*512)
with qb = i % 4. No collectives: each core computes K/V for its whole batch,
attention for its query block, and the full output projection for its rows.

Device-side layout (everything "transposed" so no on-chip transposes needed):
  - host feeds embT = embedding[b].T as bf16  [D, S]
  - Q^T, K^T computed per head-pair via block-diagonal packed weights
    (lhsT = blkdiag(W_h0^T, W_h1^T), rhs = embT rows of the pair)
  - S^T = K^T-chunks (stationary) x Q^T (moving)  ->  [t, s] layout in PSUM
  - exp on ScalarE (no max subtraction; scores are in [-11, 11])
  - V is stored [t, e] with a ones-column appended; A*V matmul then yields
    both the unnormalized output (rows 0:64) and Z = sum_t E (row 64)
  - normalization: reciprocal of Z, broadcast across partitions via a
    stride-0 DMA, multiplied in during PSUM evacuation
  - out-projection: head pairs stacked along K=128, accumulated over all
    8 pairs in PSUM; bv/bo folded into a host-precomputed bias bo2
"""

import math
from collections import deque
from contextlib import ExitStack
from functools import lru_cache

import ml_dtypes
import numpy as np

import concourse.bass as bass
import concourse.bacc as bacc
import concourse.mybir as mybir
import concourse.tile as tile

BF16 = mybir.dt.bfloat16
F32 = mybir.dt.float32
NPBF16 = ml_dtypes.bfloat16

B, S, D, H, d = 2, 2048, 1024, 16, 64
NCORES = 8
QBLKS = 4              # query blocks per batch
QB = S // QBLKS        # 512 query rows per core
NP = H // 2            # 8 head pairs
TCH = S // 128         # 16 t-chunks of 128
SCALE = 1.0 / math.sqrt(d)
EXP = mybir.ActivationFunctionType.Exp


def build_nc() -> bass.Bass:
    nc = bacc.Bacc("TRN2", target_bir_lowering=False, debug=False)

    xT_d = nc.dram_tensor("xT", [D, S], BF16, kind="ExternalInput").ap()
    xTqa_d = nc.dram_tensor("xTqa", [H, 65, QB], BF16, kind="ExternalInput").ap()
    wqa_d = nc.dram_tensor("wqa", [H, 65, 64], BF16, kind="ExternalInput").ap()
    wk_d = nc.dram_tensor("wk_blk", [NP, 128, 128], BF16, kind="ExternalInput").ap()
    wv_d = nc.dram_tensor("wv_blk", [NP, 128, 128], BF16, kind="ExternalInput").ap()
    woT_d = nc.dram_tensor("woT", [D, D], BF16, kind="ExternalInput").ap()
    bo2_d = nc.dram_tensor("bo2", [1, D], F32, kind="ExternalInput").ap()
    y_d = nc.dram_tensor("y", [QB, D], F32, kind="ExternalOutput").ap()

    rdr_d = nc.dram_tensor("rscratch", [NP, 2, QB], F32).ap()

    with ExitStack() as ctx:
        tc = ctx.enter_context(tile.TileContext(nc))
        persist = ctx.enter_context(tc.tile_pool(name="persist", bufs=1))

        wqa_sb = [persist.tile([65, 64], BF16, name=f"wqa{h}", tag=f"wqa{h}") for h in range(H)]
        wk_sb = [persist.tile([128, 128], BF16, name=f"wk{p}", tag=f"wk{p}") for p in range(NP)]
        wv_sb = [persist.tile([128, 128], BF16, name=f"wv{p}", tag=f"wv{p}") for p in range(NP)]
        xTqa_sb = [persist.tile([65, QB], BF16, name=f"xTqa{h}", tag=f"xTqa{h}") for h in range(H)]
        boB_sb = persist.tile([128, D], F32, name="boB", tag="boB")
        woT_sb = [persist.tile([128, D], BF16, name=f"woT{p}", tag=f"woT{p}") for p in range(NP)]
        qT_sb = [persist.tile([128, QB], BF16, name=f"qT{p}", tag=f"qT{p}") for p in range(NP)]
        kT_sb = [persist.tile([128, S], BF16, name=f"kT{p}", tag=f"kT{p}") for p in range(NP)]
        vv_sb = [
            [persist.tile([128, 130], BF16, name=f"vv{p}_{t}", tag=f"vv{p}_{t}") for t in range(TCH)]
            for p in range(NP)
        ]
        outT_sb = [persist.tile([128, QB], BF16, name=f"outT{p}", tag=f"outT{p}") for p in range(NP)]

        for p in range(NP):
            nc.sync.dma_start(out=wk_sb[p][:], in_=wk_d[p])
            nc.sync.dma_start(out=wv_sb[p][:], in_=wv_d[p])
            nc.sync.dma_start(out=woT_sb[p][:], in_=woT_d[p * 128 : (p + 1) * 128, :])
        for h in range(H):
            nc.sync.dma_start(out=wqa_sb[h][:], in_=wqa_d[h])
            nc.sync.dma_start(out=xTqa_sb[h][:], in_=xTqa_d[h])
        # broadcast bo2 [1, D] across all 128 partitions
        nc.sync.dma_start(
            out=boB_sb[:],
            in_=bass.AP(tensor=bo2_d.tensor, offset=bo2_d.offset, ap=[[0, 128], [1, D]]),
        )

        # ---------------- Phases 1+2: QKV + attention, interleaved ----------------
        # The attention inner loop alone leaves ~300ns PE gaps per exp-group
        # (ACT-bound), which lets the PE HAM clock-gate drop to 1.2 GHz. QKV
        # projection matmuls of pair p+2 are interleaved into the attention
        # stream of pair p as filler so the PE stays busy (and warm).
        with (
            tc.tile_pool(name="xTpool", bufs=1) as xpool,
            tc.tile_pool(name="pp1", bufs=1, space="PSUM") as pp1,
            tc.tile_pool(name="pps", bufs=2, space="PSUM") as pps,
            tc.tile_pool(name="ppav", bufs=2, space="PSUM") as ppav,
            tc.tile_pool(name="eTp", bufs=4) as eTp,
            tc.tile_pool(name="rbp", bufs=2) as rbp,
        ):
            xT_sb = [xpool.tile([128, S], BF16, name=f"xT{p}", tag=f"xT{p}") for p in range(NP)]
            for p in range(NP):
                nc.sync.dma_start(out=xT_sb[p][:], in_=xT_d[p * 128 : (p + 1) * 128, :])

            def qkv_thunks(p):
                th = []
                for dlt in range(2):
                    def _q(p=p, dlt=dlt):
                        h = 2 * p + dlt
                        pq = pp1.tile([64, QB], F32, name="pq", tag="qk")
                        nc.tensor.matmul(pq[:], wqa_sb[h][:], xTqa_sb[h][:], start=True, stop=True)
                        nc.vector.tensor_copy(qT_sb[p][dlt * 64 : (dlt + 1) * 64, :], pq[:])
                    th.append(_q)
                for ck in range(S // 512):
                    def _k(p=p, ck=ck):
                        pk = pp1.tile([128, 512], F32, name="pk", tag="qk")
                        nc.tensor.matmul(
                            pk[:],
                            wk_sb[p][:],
                            xT_sb[p][:, ck * 512 : (ck + 1) * 512],
                            start=True,
                            stop=True,
                        )
                        nc.vector.tensor_copy(kT_sb[p][:, ck * 512 : (ck + 1) * 512], pk[:])
                    th.append(_k)
                for t in range(TCH):
                    def _v(p=p, t=t):
                        pv = pp1.tile([128, 128], F32, name="pv", tag="v")
                        nc.tensor.matmul(
                            pv[:],
                            xT_sb[p][:, t * 128 : (t + 1) * 128],
                            wv_sb[p][:],
                            start=True,
                            stop=True,
                        )
                        vt = vv_sb[p][t]
                        nc.vector.tensor_copy(vt[:, 0:64], pv[:, 0:64])
                        nc.vector.tensor_copy(vt[:, 65:129], pv[:, 64:128])
                        nc.vector.memset(vt[:, 64:65], 1.0)
                        nc.vector.memset(vt[:, 129:130], 1.0)
                    th.append(_v)
                return th

            for th in qkv_thunks(0) + qkv_thunks(1):
                th()

            filler = deque()
            NG = TCH // 2
            av_of = {}
            avS_of = {}
            pending = None

            def emit_head_evac(p, dlt):
                if dlt == 0:
                    avS_of[p] = rbp.tile([128, QB], F32, name="avP", tag="avP")
                avP = avS_of[p]
                zS = rbp.tile([1, QB], F32, name=f"zS{dlt}", tag=f"zS{dlt}")
                nc.vector.tensor_copy(avP[dlt * 64 : (dlt + 1) * 64, :], av_of[(p, dlt)][0:64, :])
                nc.vector.tensor_copy(zS[:], av_of[(p, dlt)][64:65, :])
                nc.sync.dma_start(out=rdr_d[p, dlt], in_=zS[:])

            def emit_pair_evac(p):
                Zb = rbp.tile([128, QB], F32, name="Zb", tag="Zb")
                Rb = rbp.tile([128, QB], F32, name="Rb", tag="Rb")
                for dlt in range(2):
                    src_ap = rdr_d[p, dlt]
                    nc.sync.dma_start(
                        out=Zb[dlt * 64 : (dlt + 1) * 64, :],
                        in_=bass.AP(
                            tensor=src_ap.tensor,
                            offset=src_ap.offset,
                            ap=[[0, 64], [1, QB]],
                        ),
                    )
                nc.vector.reciprocal(Rb[:], Zb[:])
                nc.vector.tensor_mul(outT_sb[p][:], avS_of[p][:], Rb[:])

            def emit_av(p, dlt, g, eT):
                av = av_of[(p, dlt)]
                for j in range(2):
                    t = g * 2 + j
                    nc.tensor.matmul(
                        av[0:65, :],
                        vv_sb[p][t][:, dlt * 65 : dlt * 65 + 65],
                        eT[:, j * 512 : (j + 1) * 512],
                        start=(g == 0 and j == 0),
                        stop=(g == NG - 1 and j == 1),
                    )
                if g == NG - 1:
                    emit_head_evac(p, dlt)
                    if dlt == 1:
                        emit_pair_evac(p)

            for p in range(NP):
                if p + 2 < NP:
                    filler.extend(qkv_thunks(p + 2))
                for dlt in range(2):
                    av_of[(p, dlt)] = ppav.tile([128, QB], F32, name="av", tag="av")
                    klo = dlt * 64
                    for g in range(NG):
                        ps = pps.tile([128, 1024], F32, name="ps", tag="ps")
                        for j in range(2):
                            t = g * 2 + j
                            nc.tensor.matmul(
                                ps[:, j * 512 : (j + 1) * 512],
                                kT_sb[p][klo : klo + 64, t * 128 : (t + 1) * 128],
                                qT_sb[p][klo : klo + 64, :],
                                start=True,
                                stop=True,
                            )
                        eT = eTp.tile([128, 1024], BF16, name="eT", tag="eT")
                        nc.scalar.activation(eT[:], ps[:], EXP, scale=SCALE)
                        for _ in range(2):
                            if filler:
                                filler.popleft()()
                        if pending is not None:
                            emit_av(*pending)
                        pending = (p, dlt, g, eT)
            emit_av(*pending)

        # ---------------- Phase 3: output projection ----------------
        with (
            tc.tile_pool(name="ppy", bufs=2, space="PSUM") as ppy,
            tc.tile_pool(name="yop", bufs=3) as yop,
        ):
            for sc in range(QB // 128):
                for nk in range(D // 512):
                    py = ppy.tile([128, 512], F32, name="py", tag="py")
                    for p in range(NP):
                        nc.tensor.matmul(
                            py[:],
                            outT_sb[p][:, sc * 128 : (sc + 1) * 128],
                            woT_sb[p][:, nk * 512 : (nk + 1) * 512],
                            start=(p == 0),
                            stop=(p == NP - 1),
                        )
                    yt = yop.tile([128, 512], F32, name="yt", tag="yt")
                    nc.vector.tensor_add(yt[:], py[:], boB_sb[:, nk * 512 : (nk + 1) * 512])
                    nc.sync.dma_start(
                        out=y_d[sc * 128 : (sc + 1) * 128, nk * 512 : (nk + 1) * 512],
                        in_=yt[:],
                    )

    nc.finalize()
    return nc


@lru_cache(maxsize=1)
def _cached_nc() -> bass.Bass:
    return build_nc()


def prepare_in_maps(embedding, Wq, Wk, Wv, bq, bk, bv, Wo, bo):
    """Host-side sharding/packing. Returns per-core input maps."""
    emb = np.asarray(embedding, dtype=np.float32)
    Wq = np.asarray(Wq, dtype=np.float32)
    Wk = np.asarray(Wk, dtype=np.float32)
    Wv = np.asarray(Wv, dtype=np.float32)
    bq = np.asarray(bq, dtype=np.float32)
    bk = np.asarray(bk, dtype=np.float32)
    bv = np.asarray(bv, dtype=np.float32)
    Wo = np.asarray(Wo, dtype=np.float32)
    bo = np.asarray(bo, dtype=np.float32)

    wk_blk = np.zeros([NP, 128, 128], np.float32)
    wv_blk = np.zeros([NP, 128, 128], np.float32)
    for p in range(NP):
        h0, h1 = 2 * p, 2 * p + 1
        wk_blk[p, 0:64, 0:64] = Wk[h0].T
        wk_blk[p, 64:128, 64:128] = Wk[h1].T
        wv_blk[p, 0:64, 0:64] = Wv[h0].T
        wv_blk[p, 64:128, 64:128] = Wv[h1].T
    # per-head augmented Q weights: rows 0:64 = Wq_h^T, row 64 = bq_h
    wqa = np.zeros([H, 65, 64], np.float32)
    for h in range(H):
        wqa[h, 0:64, :] = Wq[h].T
        wqa[h, 64, :] = bq[h]

    wqa16 = wqa.astype(NPBF16)
    wk16 = wk_blk.astype(NPBF16)
    wv16 = wv_blk.astype(NPBF16)
    woT16 = np.ascontiguousarray(Wo.T).astype(NPBF16)
    bo2 = (bo + Wo @ bv.reshape(-1)).reshape(1, D).astype(np.float32)

    xT_by_b = [np.ascontiguousarray(emb[b].T).astype(NPBF16) for b in range(B)]

    in_maps = []
    for core in range(NCORES):
        b, qb = core // QBLKS, core % QBLKS
        xT = xT_by_b[b]
        # per-head augmented xTq: rows 0:64 = embT rows of head h, row 64 = ones
        xTqa = np.ones([H, 65, QB], np.float32)
        for h in range(H):
            xTqa[h, 0:64, :] = xT[h * 64 : (h + 1) * 64, qb * QB : (qb + 1) * QB]
        in_maps.append(
            dict(
                xT=xT,
                xTqa=xTqa.astype(NPBF16),
                wqa=wqa16,
                wk_blk=wk16,
                wv_blk=wv16,
                woT=woT16,
                bo2=bo2,
            )
        )
    return in_maps


def assemble(results) -> np.ndarray:
    out = np.empty([B, S, D], np.float32)
    for core in range(NCORES):
        b, qb = core // QBLKS, core % QBLKS
        out[b, qb * QB : (qb + 1) * QB, :] = results[core]["y"]
    return out


def kernel(**inputs) -> np.ndarray:
    from concourse.bass_utils import run_bass_kernel_spmd

    in_maps = prepare_in_maps(**inputs)
    nc = _cached_nc()
    res = run_bass_kernel_spmd(nc, in_maps, list(range(NCORES)))
    return assemble(res.results)
